# revision 7
# baseline (speedup 1.0000x reference)
"""Trainium2 Bass kernel for nn_MemoryQueueContrastiveLoss.

Strategy (8 NeuronCores):
  - Shard the QUEUE dimension (65536 -> 8 x 8192) across cores; replicate the
    batch features.  Each core computes partial queue negative sums
    (sum_q exp(s/t)) for ALL 1024 batch rows over its queue shard, plus the
    batch-vs-batch part for its own 128-row / 128-col shard.
  - Two ReduceScatter collectives combine the per-core partial sums so core k
    receives exactly its row-shard slice of the global negative sums.
  - Each core then computes its shard of the final loss terms
      log1p(neg * exp(-s)) = ln(exp(s) + neg) - s
    and returns per-partition partial sums; the host adds 8x[128] partials.

All transcendentals (exp/ln) run on the ACT engine, which is the bottleneck
(~2*B*Q/8 = 16.8M exps/core).  Matmuls run as float32r (full PE rate).
"""

import sys

for _p in ("/opt/trn_rl_repo",):
    if _p not in sys.path:
        sys.path.insert(0, _p)

import numpy as np

import concourse.bass as bass  # noqa: F401  (registers types)
import concourse.bacc as bacc
import concourse.mybir as mybir
from concourse import tile
from concourse import bass_utils

B = 1024          # batch
D = 128           # feature dim
Q = 65536         # queue size
NCORES = 8
QS = Q // NCORES  # 8192 queue columns per core
RT = B // 128     # 8 row tiles
INIT_TEMP = 0.07
MAX_TEMP = 0.07 * 1.3

F32 = mybir.dt.float32
F32R = mybir.dt.float32r
I32 = mybir.dt.int32
AF = mybir.ActivationFunctionType
ALU = mybir.AluOpType
AX = mybir.AxisListType

# ACT tile width for the queue exp grind: 2048 fp32 = 4 PSUM banks.
GW = 2048
NG = QS // GW     # 4 grind chunks per row tile
NMM = GW // 512   # 4 matmuls per grind chunk

# v2 dual-engine grind: 8 sub-chunks of 1024 queue cols per row tile.
# ACT consumes 5 (even row tiles) / 4 (odd) sub-chunks with exact
# exp+accumulate; DVE consumes the rest via a Schraudolph bit-trick
# (affine + f32->i32 convert writes the bit pattern of ~exp(z), then one
# fused pair-sum tensor_scalar with accum).  Engines get DISJOINT PSUM
# tiles and accumulator tiles so their streams never serialize.
SC = 1024                  # sub-chunk width
NSC = QS // SC             # 8 sub-chunks per row tile
import os as _os_cfg
_ACT_MODE = _os_cfg.environ.get("KSPLIT", "mix")
if _ACT_MODE == "act":
    ACT_SC_EVEN = tuple(range(8))
    ACT_SC_ODD = tuple(range(8))
elif _ACT_MODE == "dve":
    ACT_SC_EVEN = ()
    ACT_SC_ODD = ()
else:
    ACT_SC_EVEN = (0, 2, 4, 6, 7)
    ACT_SC_ODD = (0, 2, 4, 6)
LN2 = 0.6931471805599453
# f32 Schraudolph bias with mean-error correction (-482870 ~= -0.0576 oct)
B32_TRICK = 1064870346.0


def _f32r(ap):
    return ap.bitcast(F32R)


def build(
    eff_temp: float,
    queue_weight: float,
    n_cores: int = NCORES,
    stage: int = 8,
    bench_loops: int = 0,
):
    """Emit + compile the SPMD program (same program on all cores).

    stage (debug bisect): 1=DMA+norms, 2=+sims matmul/exp, 3=+exp accum,
    4=+full phase B, 5=+text grind, 6=+RS2, 7=+vision grind+RS1, 8=full.
    """
    scale_b = 1.0 / eff_temp            # batch sims logits scale
    scale_q = queue_weight / eff_temp   # queue logits scale

    nc = bacc.Bacc(
        "TRN2", target_bir_lowering=False, debug=False, num_devices=n_cores
    )

    # ---- kernel I/O (per core) ----
    vfT_d = nc.dram_tensor("vfT", [D, B], F32R, kind="ExternalInput")
    tfT_d = nc.dram_tensor("tfT", [D, B], F32R, kind="ExternalInput")
    vfrkT_d = nc.dram_tensor("vf_rkT", [D, 128], F32R, kind="ExternalInput")
    tfrkT_d = nc.dram_tensor("tf_rkT", [D, 128], F32R, kind="ExternalInput")
    mid_d = nc.dram_tensor("mid", [128, B], F32, kind="ExternalInput")
    midrk_d = nc.dram_tensor("mid_rk", [128, 1], F32, kind="ExternalInput")
    tq_d = nc.dram_tensor("tq", [D, QS], F32R, kind="ExternalInput")
    vq_d = nc.dram_tensor("vq", [D, QS], F32R, kind="ExternalInput")
    out_d = nc.dram_tensor("partials", [128, 3], F32, kind="ExternalOutput")

    # ---- collective buffers (internal DRAM) ----
    # cc2: qsum_v partials, laid out [row_tile, lane] so ReduceScatter hands
    # core k the summed block for its own row shard.
    cc2_in = nc.dram_tensor("cc2_in", [RT, 128], F32)
    cc2_out = nc.dram_tensor("cc2_out", [1, 128], F32)
    # cc1: [row_tile, 2, lane] = (qsum_t, batch colsum) partials.
    cc1_in = nc.dram_tensor("cc1_in", [RT, 2, 128], F32)
    cc1_out = nc.dram_tensor("cc1_out", [2, 128], F32)

    rg = [list(range(n_cores))]

    with tile.TileContext(nc) as tc:
        with tc.tile_pool(name="sb", bufs=1) as sb:
            # persistent SBUF tiles
            vfT = sb.tile([D, B], F32R, tag="vfT")
            tfT = sb.tile([D, B], F32R, tag="tfT")
            vfrkT = sb.tile([D, 128], F32R, tag="vfrkT")
            tfrkT = sb.tile([D, 128], F32R, tag="tfrkT")
            midb = sb.tile([128, B], F32, tag="midb")
            midrk = sb.tile([128, 1], F32, tag="midrk")
            tq_sb = sb.tile([D, QS], F32R, tag="tq")
            vq_sb = sb.tile([D, QS], F32R, tag="vq")
            mask = sb.tile([128, B], F32, tag="mask")
            sqbuf = sb.tile([128, B], F32, tag="sqbuf")
            lnbuf = sb.tile([1, B], F32, tag="lnbuf")
            rnbuf = sb.tile([1, B], F32, tag="rnbuf")
            ones = sb.tile([128, 1], F32, tag="ones")
            nones = sb.tile([128, 1], F32, tag="nones")
            ones1 = sb.tile([1, 128], F32R, tag="ones1")
            ones1f = sb.tile([1, 128], F32, tag="ones1f")
            ones_r = sb.tile([128, 1], F32R, tag="ones_r")
            E_r = sb.tile([128, B], F32, tag="E_r")
            ET_c = sb.tile([128, B], F32, tag="ET_c")
            rsumE = sb.tile([128, 1], F32, tag="rsumE")
            possum = sb.tile([128, 1], F32, tag="possum")
            rnm = sb.tile([128, 1], F32, tag="rnm")
            cs_sb = sb.tile([1, B], F32, tag="cs_sb")
            np_rows = sb.tile([128, 1], F32, tag="np_rows")
            qsum_v = sb.tile([128, RT], F32, tag="qsum_v")
            qsum_t = sb.tile([128, RT], F32, tag="qsum_t")
            p2trash = sb.tile([128, 2048], F32, tag="p2trash")
            trashB = sb.tile([128, B], F32, tag="trashB")
            qvt = sb.tile([128, 1], F32, tag="qvt")
            qtt = sb.tile([128, 1], F32, tag="qtt")
            cst = sb.tile([128, 1], F32, tag="cst")
            negv = sb.tile([128, 1], F32, tag="negv")
            negt = sb.tile([128, 1], F32, tag="negt")
            lsum_v = sb.tile([128, 1], F32, tag="lsum_v")
            lsum_t = sb.tile([128, 1], F32, tag="lsum_t")
            ssum_v = sb.tile([128, 1], F32, tag="ssum_v")
            ssum_t = sb.tile([128, 1], F32, tag="ssum_t")
            lv = sb.tile([128, 1], F32, tag="lv")
            lt = sb.tile([128, 1], F32, tag="lt")

            # ---------- input DMAs ----------
            nc.sync.dma_start(out=vfT[:, :], in_=vfT_d.ap()[:, :])
            nc.sync.dma_start(out=tfT[:, :], in_=tfT_d.ap()[:, :])
            nc.sync.dma_start(out=vfrkT[:, :], in_=vfrkT_d.ap()[:, :])
            nc.sync.dma_start(out=tfrkT[:, :], in_=tfrkT_d.ap()[:, :])
            nc.sync.dma_start(out=midb[:, :], in_=mid_d.ap()[:, :])
            nc.sync.dma_start(out=midrk[:, :], in_=midrk_d.ap()[:, :])
            # queue shards, chunked so compute can start early
            for c in range(NG):
                cs_ = slice(c * GW, (c + 1) * GW)
                nc.sync.dma_start(out=tq_sb[:, cs_], in_=tq_d.ap()[:, cs_])
            for c in range(NG):
                cs_ = slice(c * GW, (c + 1) * GW)
                nc.sync.dma_start(out=vq_sb[:, cs_], in_=vq_d.ap()[:, cs_])

            nc.vector.memset(ones[:, :], 1.0)
            nc.vector.memset(nones[:, :], -1.0)
            nc.vector.memset(ones1f[:, :], 1.0)
            nc.vector.tensor_copy(ones1[:, :], ones1f[:, :])
            nc.vector.tensor_copy(ones_r[:, :], ones[:, :])

            # ---------- phase A: l2-normalize features (in place) ----------
            def norm_chain(xT, n, psA):
                nc.vector.tensor_mul(_f32r(sqbuf[:, :n]), xT[:, :], xT[:, :])
                n2 = psA.tile([1, B], F32, tag="n2")
                for j in range(0, n, 512):
                    nc.tensor.matmul(
                        n2[:, j : j + 512],
                        ones_r[:, :],
                        _f32r(sqbuf[:, j : j + 512]),
                        start=True,
                        stop=True,
                    )
                # rnorm = exp(-0.5 * ln(norm2))  (avoids sqrt table load)
                nc.scalar.activation(lnbuf[:, :n], n2[:, :n], AF.Ln)
                nc.scalar.activation(
                    _f32r(rnbuf[:, :n]), lnbuf[:, :n], AF.Exp, scale=-0.5
                )
                # broadcast rnorm across partitions via PE: ones1^T @ rnorm_row
                rb = psA.tile([128, B], F32, tag="rb")
                for j in range(0, n, 512):
                    nc.tensor.matmul(
                        rb[:, j : j + 512],
                        ones1[0:1, :],
                        _f32r(rnbuf[0:1, j : j + 512]),
                        start=True,
                        stop=True,
                    )
                # write the normalized features as float32r so the verifier
                # accepts them as fp32r-matmul inputs
                nc.vector.tensor_mul(_f32r(xT[:, :]), xT[:, :], rb[:, :n])

            with tc.tile_pool(name="psA", bufs=2, space="PSUM") as psA:
                norm_chain(vfT, B, psA)   # vision first: text-queue grind needs it
                norm_chain(tfT, B, psA)
                norm_chain(vfrkT, 128, psA)
                norm_chain(tfrkT, 128, psA)

            # match mask for this core's row/col shard: mask[p, j] =
            # (mid[rk_p] == mid[j])
            nc.vector.tensor_scalar(
                mask[:, :], midb[:, :], midrk[:, 0:1], None, ALU.is_equal
            )
            nc.vector.reduce_sum(np_rows[:, :], mask[:, :], axis=AX.X)

            # ---------- phase B: batch sims for own shard ----------
            if stage >= 2:
                with tc.tile_pool(name="psB", bufs=1, space="PSUM") as psB:
                    sims_r = psB.tile([128, B], F32, tag="sims_r")
                    simsT_c = psB.tile([128, B], F32, tag="simsT_c")
                    cs_ps = psB.tile([1, B], F32, tag="cs_ps")
                    for j in range(0, B, 512):
                        nc.tensor.matmul(
                            sims_r[:, j : j + 512],
                            _f32r(vfrkT[:, :]),
                            _f32r(tfT[:, j : j + 512]),
                            start=True,
                            stop=True,
                        )
                    nc.scalar.activation(
                        E_r[:, :],
                        sims_r[:, :],
                        AF.Exp,
                        scale=scale_b,
                        accum_out=rsumE[:, :] if stage >= 3 else None,
                    )
                    for j in range(0, B, 512):
                        nc.tensor.matmul(
                            simsT_c[:, j : j + 512],
                            _f32r(tfrkT[:, :]),
                            _f32r(vfT[:, j : j + 512]),
                            start=True,
                            stop=True,
                        )
                    nc.scalar.activation(
                        ET_c[:, :], simsT_c[:, :], AF.Exp, scale=scale_b
                    )

                    import os as _os

                    _sub = int(_os.environ.get("KSUB", "9"))
                    if stage >= 4 and _sub >= 1:
                        # Em = E_r * mask ; possum = rowsum(Em)
                        nc.vector.tensor_mul(trashB[:, :], E_r[:, :], mask[:, :])
                        nc.vector.reduce_sum(possum[:, :], trashB[:, :], axis=AX.X)
                        nc.vector.tensor_sub(rnm[:, :], rsumE[:, :], possum[:, :])
                    if stage >= 4 and _sub >= 2:
                        # batch colsums of non-matching exp(sims)
                        for j in range(0, B, 512):
                            nc.tensor.matmul(
                                cs_ps[:, j : j + 512],
                                ones[:, :],
                                E_r[:, j : j + 512],
                                start=True,
                                stop=False,
                            )
                            nc.tensor.matmul(
                                cs_ps[:, j : j + 512],
                                nones[:, :],
                                trashB[:, j : j + 512],
                                start=False,
                                stop=True,
                            )
                        nc.vector.tensor_copy(cs_sb[:, :], cs_ps[:, :])
                    else:
                        nc.vector.tensor_copy(cs_sb[:, :], E_r[0:1, :])
                    # masked sims sums (independent of the collectives) are
                    # computed here, off the post-RS critical path
                    nc.vector.tensor_mul(trashB[:, :], sims_r[:, :], mask[:, :])
                    nc.vector.reduce_sum(ssum_v[:, :], trashB[:, :], axis=AX.X)
                    nc.vector.tensor_scalar(
                        ssum_v[:, :], ssum_v[:, :], scale_b, None, ALU.mult
                    )
                    nc.vector.tensor_mul(trashB[:, :], simsT_c[:, :], mask[:, :])
                    nc.vector.reduce_sum(ssum_t[:, :], trashB[:, :], axis=AX.X)
                    nc.vector.tensor_scalar(
                        ssum_t[:, :], ssum_t[:, :], scale_b, None, ALU.mult
                    )

            # ---------- queue grind ----------
            # Per row tile: 4 chunks of 2048 matmul columns land in PSUM
            # (double buffered).  3 chunks are copied by DVE into an SBUF
            # staging tile and exp'd in ONE wide ACT instruction (amortizes
            # the per-instruction ACT overhead); the 4th chunk is exp'd
            # directly from PSUM (in place) so ACT and DVE loads balance
            # (ACT ~0.88ns/elem staged + 1 chunk direct vs DVE 1.13ns/elem
            # on the staged 3/4 of the data).

            A32_trick = (8388608.0 / LN2) * scale_q

            def grind_direct(queue_sb, lhsT, qsum, pg, est_pool, cc_ap=None):
                # v2: dual-engine grind.  Per row tile, 8 sub-chunks of 1024
                # matmul cols land in their own [128,1024] PSUM tiles
                # (bufs=4); ~56% drain on ACT (exact exp, accum in accA),
                # ~44% on DVE (trick-convert into an f32 stage, then one
                # fused pair-sum with accum into accD).
                for r in range(RT):
                    lhs = _f32r(lhsT[:, r * 128 : (r + 1) * 128])
                    act_scs = ACT_SC_EVEN if r % 2 == 0 else ACT_SC_ODD
                    ndve = NSC - len(act_scs)
                    stage = accA = accD = None
                    if ndve:
                        stage = est_pool.tile([128, 4096], F32, tag="stage")
                        accD = est_pool.tile([128, 1], F32, tag="accD")
                    if act_scs:
                        accA = est_pool.tile([128, 8], F32, tag="accA")
                    rsumA = est_pool.tile([128, 1], F32, tag="rsumA")
                    di = 0
                    for sc in range(NSC):
                        ps = pg.tile([128, SC], F32, tag="gps")
                        for j in range(2):
                            col = sc * SC + j * 512
                            nc.tensor.matmul(
                                ps[:, j * 512 : (j + 1) * 512],
                                lhs,
                                queue_sb[:, col : col + 512],
                                start=True,
                                stop=True,
                            )
                        if sc in act_scs:
                            k = act_scs.index(sc)
                            nc.scalar.activation(
                                ps[:, :],
                                ps[:, :],
                                AF.Exp,
                                scale=scale_q,
                                accum_out=accA[:, k : k + 1],
                            )
                        else:
                            nc.vector.tensor_scalar(
                                stage[:, di * SC : (di + 1) * SC].bitcast(I32),
                                ps[:, :],
                                A32_trick,
                                B32_TRICK,
                                ALU.mult,
                                ALU.add,
                            )
                            di += 1
                    if ndve:
                        used = di * SC
                        h = used // 2
                        nc.vector.scalar_tensor_tensor(
                            p2trash[:, 0:h],
                            stage[:, 0:h],
                            1.0,
                            stage[:, h:used],
                            ALU.mult,
                            ALU.add,
                            accum_out=accD[:, :],
                        )
                    if act_scs and ndve:
                        nc.vector.reduce_sum(
                            rsumA[:, :], accA[:, 0 : len(act_scs)], axis=AX.X
                        )
                        nc.vector.tensor_add(
                            qsum[:, r : r + 1], rsumA[:, :], accD[:, :]
                        )
                    elif act_scs:
                        nc.vector.reduce_sum(
                            qsum[:, r : r + 1], accA[:, 0 : len(act_scs)],
                            axis=AX.X,
                        )
                    else:
                        nc.vector.tensor_copy(qsum[:, r : r + 1], accD[:, :])
                    if cc_ap is not None:
                        # stream this row tile's partial sums out immediately so
                        # the ReduceScatter can start right after the last exp
                        nc.sync.dma_start(out=cc_ap[r], in_=qsum[:, r : r + 1])

            grind = grind_direct

            if bench_loops > 0:
                # benchmark mode: repeat both grinds inside a HW loop; the
                # grinds are idempotent so results stay correct.
                assert stage >= 8
                with (
                    tc.tile_pool(name="pgb", bufs=4, space="PSUM") as pg,
                    tc.tile_pool(name="estb", bufs=2) as estp,
                ):
                    with tc.For_i(0, bench_loops, 1):
                        grind(tq_sb, vfT, qsum_v, pg, estp)
                        grind(vq_sb, tfT, qsum_t, pg, estp)
            elif stage >= 5:
                # text queue -> qsum_v (feeds RS2)
                with (
                    tc.tile_pool(name="pgv", bufs=4, space="PSUM") as pg,
                    tc.tile_pool(name="estv", bufs=2) as estp,
                ):
                    cc2aps = (
                        [cc2_in.ap()[r, :] for r in range(RT)]
                        if stage >= 6
                        else None
                    )
                    grind(tq_sb, vfT, qsum_v, pg, estp, cc2aps)

            if stage >= 6:
                nc.gpsimd.collective_compute(
                    "ReduceScatter",
                    ALU.add,
                    replica_groups=rg,
                    ins=[cc2_in.ap().opt()],
                    outs=[cc2_out.ap().opt()],
                )

            if stage >= 7:
                # vision queue -> qsum_t (feeds RS1)
                if bench_loops == 0:
                    with (
                        tc.tile_pool(name="pgt", bufs=4, space="PSUM") as pg,
                        tc.tile_pool(name="estt", bufs=2) as estp,
                    ):
                        cc1aps = [cc1_in.ap()[r, 0, :] for r in range(RT)]
                        grind(vq_sb, tfT, qsum_t, pg, estp, cc1aps)
                for r in range(RT):
                    if bench_loops != 0:
                        nc.sync.dma_start(
                            out=cc1_in.ap()[r, 0, :], in_=qsum_t[:, r : r + 1]
                        )
                    nc.sync.dma_start(
                        out=cc1_in.ap()[r, 1, :],
                        in_=cs_sb[0:1, r * 128 : (r + 1) * 128],
                    )
                nc.gpsimd.collective_compute(
                    "ReduceScatter",
                    ALU.add,
                    replica_groups=rg,
                    ins=[cc1_in.ap().opt()],
                    outs=[cc1_out.ap().opt()],
                )

            if stage >= 8:
                # ---------- phase D: loss terms for own shard ----------
                with tc.tile_pool(name="psD", bufs=1, space="PSUM") as psD:
                    # v2t: rows shard.  neg_v = batch-nonmatch rowsum + queue
                    nc.sync.dma_start(out=qvt[:, :], in_=cc2_out.ap()[0, :])
                    nc.vector.tensor_add(negv[:, :], rnm[:, :], qvt[:, :])
                    nc.scalar.activation(
                        _f32r(sqbuf[:, :]), E_r[:, :], AF.Ln, bias=negv[:, 0:1]
                    )
                    nc.vector.tensor_mul(trashB[:, :], sqbuf[:, :], mask[:, :])
                    nc.vector.reduce_sum(lsum_v[:, :], trashB[:, :], axis=AX.X)
                    nc.vector.tensor_sub(lv[:, :], lsum_v[:, :], ssum_v[:, :])

                    # t2v: cols shard.  neg_t = batch colsum + queue sum
                    nc.sync.dma_start(out=cst[:, :], in_=cc1_out.ap()[1, :])
                    nc.sync.dma_start(out=qtt[:, :], in_=cc1_out.ap()[0, :])
                    nc.vector.tensor_add(negt[:, :], cst[:, :], qtt[:, :])
                    nc.scalar.activation(
                        _f32r(sqbuf[:, :]), ET_c[:, :], AF.Ln, bias=negt[:, 0:1]
                    )
                    nc.vector.tensor_mul(trashB[:, :], sqbuf[:, :], mask[:, :])
                    nc.vector.reduce_sum(lsum_t[:, :], trashB[:, :], axis=AX.X)
                    nc.vector.tensor_sub(lt[:, :], lsum_t[:, :], ssum_t[:, :])

                # ---------- outputs ----------
                nc.sync.dma_start(out=out_d.ap()[:, 0:1], in_=lv[:, :])
                nc.sync.dma_start(out=out_d.ap()[:, 1:2], in_=lt[:, :])
                nc.sync.dma_start(out=out_d.ap()[:, 2:3], in_=np_rows[:, :])
            else:
                # debug stages: emit whatever is defined
                nc.sync.dma_start(out=out_d.ap()[:, 0:1], in_=np_rows[:, :])
                src1 = E_r if stage >= 2 else np_rows
                nc.sync.dma_start(out=out_d.ap()[:, 1:2], in_=src1[:, 0:1])
                src2 = qsum_v if stage >= 5 else np_rows
                nc.sync.dma_start(out=out_d.ap()[:, 2:3], in_=src2[:, 0:1])

    nc.compile()
    return nc


def schedule_scalars(fill_level: int):
    fill_ratio = min(int(fill_level), Q) / Q
    eff_temp = MAX_TEMP - (MAX_TEMP - INIT_TEMP) * fill_ratio
    if fill_ratio >= 0.95:
        eff_temp = INIT_TEMP
    queue_weight = min(1.0, fill_ratio * 1.5)
    if fill_ratio < 0.2:
        queue_weight = fill_ratio * 0.5
    return eff_temp, queue_weight


def make_in_maps(
    vision_features, text_features, match_ids, vision_queue, text_queue
):
    vf = np.asarray(vision_features, dtype=np.float32)
    tf_ = np.asarray(text_features, dtype=np.float32)
    vq = np.asarray(vision_queue, dtype=np.float32)
    tq = np.asarray(text_queue, dtype=np.float32)
    mid = np.asarray(match_ids).astype(np.float32)

    vfT = np.ascontiguousarray(vf.T)
    tfT = np.ascontiguousarray(tf_.T)
    mid_bcast = np.ascontiguousarray(np.broadcast_to(mid.reshape(1, B), (128, B)))

    in_maps = []
    for k in range(NCORES):
        rk = slice(k * 128, (k + 1) * 128)
        qs = slice(k * QS, (k + 1) * QS)
        in_maps.append(
            {
                "vfT": vfT,
                "tfT": tfT,
                "vf_rkT": np.ascontiguousarray(vf[rk].T),
                "tf_rkT": np.ascontiguousarray(tf_[rk].T),
                "mid": mid_bcast,
                "mid_rk": np.ascontiguousarray(mid[rk].reshape(128, 1)),
                "tq": np.ascontiguousarray(tq[:, qs]),
                "vq": np.ascontiguousarray(vq[:, qs]),
            }
        )
    return in_maps


def combine_partials(partials_list):
    """partials_list: NCORES arrays of [128, 3] -> scalar loss (fp32)."""
    P = np.stack([np.asarray(p, dtype=np.float64) for p in partials_list])
    s = P.sum(axis=(0, 1))  # [3] = (v2t, t2v, num_pos)
    loss = (s[0] / s[2] + s[1] / s[2]) / 2.0
    return np.float32(loss)


_NC_CACHE: dict = {}


def _get_compiled(eff_temp: float, queue_weight: float, stage: int = 8):
    key = (round(eff_temp, 9), round(queue_weight, 9), stage)
    if key not in _NC_CACHE:
        _NC_CACHE[key] = build(eff_temp, queue_weight, stage=stage)
    return _NC_CACHE[key]


def kernel(
    vision_features,
    text_features,
    match_ids,
    vision_queue,
    text_queue,
    fill_level,
    **_ignored,
):
    eff_temp, queue_weight = schedule_scalars(fill_level)
    nc = _get_compiled(eff_temp, queue_weight)
    in_maps = make_in_maps(
        vision_features, text_features, match_ids, vision_queue, text_queue
    )
    res = bass_utils.run_bass_kernel_spmd(
        nc, in_maps, core_ids=list(range(NCORES))
    )
    return combine_partials([r["partials"] for r in res.results])



# revision 8
# speedup vs baseline: 1.2762x; 1.2762x over previous
"""Trainium2 Bass kernel for nn_MemoryQueueContrastiveLoss.

Strategy (8 NeuronCores):
  - Shard the QUEUE dimension (65536 -> 8 x 8192) across cores; replicate the
    batch features.  Each core computes partial queue negative sums
    (sum_q exp(s/t)) for ALL 1024 batch rows over its queue shard, plus the
    batch-vs-batch part for its own 128-row / 128-col shard.
  - Two ReduceScatter collectives combine the per-core partial sums so core k
    receives exactly its row-shard slice of the global negative sums.
  - Each core then computes its shard of the final loss terms
      log1p(neg * exp(-s)) = ln(exp(s) + neg) - s
    and returns per-partition partial sums; the host adds 8x[128] partials.

All transcendentals (exp/ln) run on the ACT engine, which is the bottleneck
(~2*B*Q/8 = 16.8M exps/core).  Matmuls run as float32r (full PE rate).
"""

import sys

for _p in ("/opt/trn_rl_repo",):
    if _p not in sys.path:
        sys.path.insert(0, _p)

import numpy as np

import concourse.bass as bass  # noqa: F401  (registers types)
import concourse.bacc as bacc
import concourse.mybir as mybir
from concourse import tile
from concourse import bass_utils

B = 1024          # batch
D = 128           # feature dim
Q = 65536         # queue size
NCORES = 8
QS = Q // NCORES  # 8192 queue columns per core
RT = B // 128     # 8 row tiles
INIT_TEMP = 0.07
MAX_TEMP = 0.07 * 1.3

F32 = mybir.dt.float32
F32R = mybir.dt.float32r
I32 = mybir.dt.int32
AF = mybir.ActivationFunctionType
ALU = mybir.AluOpType
AX = mybir.AxisListType

# ACT tile width for the queue exp grind: 2048 fp32 = 4 PSUM banks.
GW = 2048
NG = QS // GW     # 4 grind chunks per row tile
NMM = GW // 512   # 4 matmuls per grind chunk

# v2 dual-engine grind: 8 sub-chunks of 1024 queue cols per row tile.
# ACT consumes 5 (even row tiles) / 4 (odd) sub-chunks with exact
# exp+accumulate; DVE consumes the rest via a Schraudolph bit-trick
# (affine + f32->i32 convert writes the bit pattern of ~exp(z), then one
# fused pair-sum tensor_scalar with accum).  Engines get DISJOINT PSUM
# tiles and accumulator tiles so their streams never serialize.
SC = 1024                  # sub-chunk width
NSC = QS // SC             # 8 sub-chunks per row tile
import os as _os_cfg
_ACT_MODE = _os_cfg.environ.get("KSPLIT", "mix")
if _ACT_MODE == "act":
    ACT_SC_EVEN = tuple(range(8))
    ACT_SC_ODD = tuple(range(8))
elif _ACT_MODE == "dve":
    ACT_SC_EVEN = ()
    ACT_SC_ODD = ()
else:
    ACT_SC_EVEN = (0, 2, 4, 6, 7)
    ACT_SC_ODD = (0, 2, 4, 6)
LN2 = 0.6931471805599453
# f32 Schraudolph bias with mean-error correction (-482870 ~= -0.0576 oct)
B32_TRICK = 1064870346.0


def _f32r(ap):
    return ap.bitcast(F32R)


def build(
    eff_temp: float,
    queue_weight: float,
    n_cores: int = NCORES,
    stage: int = 8,
    bench_loops: int = 0,
):
    """Emit + compile the SPMD program (same program on all cores).

    stage (debug bisect): 1=DMA+norms, 2=+sims matmul/exp, 3=+exp accum,
    4=+full phase B, 5=+text grind, 6=+RS2, 7=+vision grind+RS1, 8=full.
    """
    scale_b = 1.0 / eff_temp            # batch sims logits scale
    scale_q = queue_weight / eff_temp   # queue logits scale

    nc = bacc.Bacc(
        "TRN2", target_bir_lowering=False, debug=False, num_devices=n_cores
    )

    # ---- kernel I/O (per core) ----
    vfT_d = nc.dram_tensor("vfT", [D, B], F32R, kind="ExternalInput")
    tfT_d = nc.dram_tensor("tfT", [D, B], F32R, kind="ExternalInput")
    vfrkT_d = nc.dram_tensor("vf_rkT", [D, 128], F32R, kind="ExternalInput")
    tfrkT_d = nc.dram_tensor("tf_rkT", [D, 128], F32R, kind="ExternalInput")
    mid_d = nc.dram_tensor("mid", [128, B], F32, kind="ExternalInput")
    midrk_d = nc.dram_tensor("mid_rk", [128, 1], F32, kind="ExternalInput")
    tq_d = nc.dram_tensor("tq", [D, QS], F32R, kind="ExternalInput")
    vq_d = nc.dram_tensor("vq", [D, QS], F32R, kind="ExternalInput")
    out_d = nc.dram_tensor("partials", [128, 3], F32, kind="ExternalOutput")

    # ---- collective buffers (internal DRAM) ----
    # cc2: qsum_v partials, laid out [row_tile, lane] so ReduceScatter hands
    # core k the summed block for its own row shard.
    cc2_in = nc.dram_tensor("cc2_in", [RT, 128], F32)
    cc2_out = nc.dram_tensor("cc2_out", [1, 128], F32)
    # cc1: [row_tile, 2, lane] = (qsum_t, batch colsum) partials.
    cc1_in = nc.dram_tensor("cc1_in", [RT, 2, 128], F32)
    cc1_out = nc.dram_tensor("cc1_out", [2, 128], F32)

    rg = [list(range(n_cores))]

    with tile.TileContext(nc) as tc:
        with tc.tile_pool(name="sb", bufs=1) as sb:
            # persistent SBUF tiles
            vfT = sb.tile([D, B], F32R, tag="vfT")
            tfT = sb.tile([D, B], F32R, tag="tfT")
            vfrkT = sb.tile([D, 128], F32R, tag="vfrkT")
            tfrkT = sb.tile([D, 128], F32R, tag="tfrkT")
            midb = sb.tile([128, B], F32, tag="midb")
            midrk = sb.tile([128, 1], F32, tag="midrk")
            tq_sb = sb.tile([D, QS], F32R, tag="tq")
            vq_sb = sb.tile([D, QS], F32R, tag="vq")
            mask = sb.tile([128, B], F32, tag="mask")
            sqbuf = sb.tile([128, B], F32, tag="sqbuf")
            lnbuf = sb.tile([1, B], F32, tag="lnbuf")
            rnbuf = sb.tile([1, B], F32, tag="rnbuf")
            ones = sb.tile([128, 1], F32, tag="ones")
            nones = sb.tile([128, 1], F32, tag="nones")
            ones1 = sb.tile([1, 128], F32R, tag="ones1")
            ones1f = sb.tile([1, 128], F32, tag="ones1f")
            ones_r = sb.tile([128, 1], F32R, tag="ones_r")
            E_r = sb.tile([128, B], F32, tag="E_r")
            ET_c = sb.tile([128, B], F32, tag="ET_c")
            rsumE = sb.tile([128, 1], F32, tag="rsumE")
            possum = sb.tile([128, 1], F32, tag="possum")
            rnm = sb.tile([128, 1], F32, tag="rnm")
            cs_sb = sb.tile([1, B], F32, tag="cs_sb")
            np_rows = sb.tile([128, 1], F32, tag="np_rows")
            qsum_v = sb.tile([128, RT], F32, tag="qsum_v")
            qsum_t = sb.tile([128, RT], F32, tag="qsum_t")
            p2trash = sb.tile([128, 2048], F32, tag="p2trash")
            accAg_v = sb.tile([128, RT * 8], F32, tag="accAg_v")
            accDg_v = sb.tile([128, RT], F32, tag="accDg_v")
            accAg_t = sb.tile([128, RT * 8], F32, tag="accAg_t")
            accDg_t = sb.tile([128, RT], F32, tag="accDg_t")
            trashB = sb.tile([128, B], F32, tag="trashB")
            qvt = sb.tile([128, 1], F32, tag="qvt")
            qtt = sb.tile([128, 1], F32, tag="qtt")
            cst = sb.tile([128, 1], F32, tag="cst")
            negv = sb.tile([128, 1], F32, tag="negv")
            negt = sb.tile([128, 1], F32, tag="negt")
            lsum_v = sb.tile([128, 1], F32, tag="lsum_v")
            lsum_t = sb.tile([128, 1], F32, tag="lsum_t")
            ssum_v = sb.tile([128, 1], F32, tag="ssum_v")
            ssum_t = sb.tile([128, 1], F32, tag="ssum_t")
            lv = sb.tile([128, 1], F32, tag="lv")
            lt = sb.tile([128, 1], F32, tag="lt")

            # ---------- input DMAs ----------
            nc.sync.dma_start(out=vfT[:, :], in_=vfT_d.ap()[:, :])
            nc.sync.dma_start(out=tfT[:, :], in_=tfT_d.ap()[:, :])
            nc.sync.dma_start(out=vfrkT[:, :], in_=vfrkT_d.ap()[:, :])
            nc.sync.dma_start(out=tfrkT[:, :], in_=tfrkT_d.ap()[:, :])
            nc.sync.dma_start(out=midb[:, :], in_=mid_d.ap()[:, :])
            nc.sync.dma_start(out=midrk[:, :], in_=midrk_d.ap()[:, :])
            # queue shards, chunked so compute can start early
            for c in range(NG):
                cs_ = slice(c * GW, (c + 1) * GW)
                nc.sync.dma_start(out=tq_sb[:, cs_], in_=tq_d.ap()[:, cs_])
            for c in range(NG):
                cs_ = slice(c * GW, (c + 1) * GW)
                nc.sync.dma_start(out=vq_sb[:, cs_], in_=vq_d.ap()[:, cs_])

            nc.vector.memset(accAg_v[:, :], 0.0)
            nc.vector.memset(accDg_v[:, :], 0.0)
            nc.vector.memset(accAg_t[:, :], 0.0)
            nc.vector.memset(accDg_t[:, :], 0.0)
            nc.vector.memset(ones[:, :], 1.0)
            nc.vector.memset(nones[:, :], -1.0)
            nc.vector.memset(ones1f[:, :], 1.0)
            nc.vector.tensor_copy(ones1[:, :], ones1f[:, :])
            nc.vector.tensor_copy(ones_r[:, :], ones[:, :])

            # ---------- phase A: l2-normalize features (in place) ----------
            def norm_chain(xT, n, psA):
                nc.vector.tensor_mul(_f32r(sqbuf[:, :n]), xT[:, :], xT[:, :])
                n2 = psA.tile([1, B], F32, tag="n2")
                for j in range(0, n, 512):
                    nc.tensor.matmul(
                        n2[:, j : j + 512],
                        ones_r[:, :],
                        _f32r(sqbuf[:, j : j + 512]),
                        start=True,
                        stop=True,
                    )
                # rnorm = exp(-0.5 * ln(norm2))  (avoids sqrt table load)
                nc.scalar.activation(lnbuf[:, :n], n2[:, :n], AF.Ln)
                nc.scalar.activation(
                    _f32r(rnbuf[:, :n]), lnbuf[:, :n], AF.Exp, scale=-0.5
                )
                # broadcast rnorm across partitions via PE: ones1^T @ rnorm_row
                rb = psA.tile([128, B], F32, tag="rb")
                for j in range(0, n, 512):
                    nc.tensor.matmul(
                        rb[:, j : j + 512],
                        ones1[0:1, :],
                        _f32r(rnbuf[0:1, j : j + 512]),
                        start=True,
                        stop=True,
                    )
                # write the normalized features as float32r so the verifier
                # accepts them as fp32r-matmul inputs
                nc.vector.tensor_mul(_f32r(xT[:, :]), xT[:, :], rb[:, :n])

            with tc.tile_pool(name="psA", bufs=2, space="PSUM") as psA:
                norm_chain(vfT, B, psA)   # vision first: text-queue grind needs it
                norm_chain(tfT, B, psA)
                norm_chain(vfrkT, 128, psA)
                norm_chain(tfrkT, 128, psA)

            # match mask for this core's row/col shard: mask[p, j] =
            # (mid[rk_p] == mid[j])
            nc.vector.tensor_scalar(
                mask[:, :], midb[:, :], midrk[:, 0:1], None, ALU.is_equal
            )
            nc.vector.reduce_sum(np_rows[:, :], mask[:, :], axis=AX.X)

            # ---------- phase B: batch sims for own shard ----------
            if stage >= 2:
                with tc.tile_pool(name="psB", bufs=1, space="PSUM") as psB:
                    sims_r = psB.tile([128, B], F32, tag="sims_r")
                    simsT_c = psB.tile([128, B], F32, tag="simsT_c")
                    cs_ps = psB.tile([1, B], F32, tag="cs_ps")
                    for j in range(0, B, 512):
                        nc.tensor.matmul(
                            sims_r[:, j : j + 512],
                            _f32r(vfrkT[:, :]),
                            _f32r(tfT[:, j : j + 512]),
                            start=True,
                            stop=True,
                        )
                    nc.scalar.activation(
                        E_r[:, :],
                        sims_r[:, :],
                        AF.Exp,
                        scale=scale_b,
                        accum_out=rsumE[:, :] if stage >= 3 else None,
                    )
                    for j in range(0, B, 512):
                        nc.tensor.matmul(
                            simsT_c[:, j : j + 512],
                            _f32r(tfrkT[:, :]),
                            _f32r(vfT[:, j : j + 512]),
                            start=True,
                            stop=True,
                        )
                    nc.scalar.activation(
                        ET_c[:, :], simsT_c[:, :], AF.Exp, scale=scale_b
                    )

                    import os as _os

                    _sub = int(_os.environ.get("KSUB", "9"))
                    if stage >= 4 and _sub >= 1:
                        # Em = E_r * mask ; possum = rowsum(Em)
                        nc.vector.tensor_mul(trashB[:, :], E_r[:, :], mask[:, :])
                        nc.vector.reduce_sum(possum[:, :], trashB[:, :], axis=AX.X)
                        nc.vector.tensor_sub(rnm[:, :], rsumE[:, :], possum[:, :])
                    if stage >= 4 and _sub >= 2:
                        # batch colsums of non-matching exp(sims)
                        for j in range(0, B, 512):
                            nc.tensor.matmul(
                                cs_ps[:, j : j + 512],
                                ones[:, :],
                                E_r[:, j : j + 512],
                                start=True,
                                stop=False,
                            )
                            nc.tensor.matmul(
                                cs_ps[:, j : j + 512],
                                nones[:, :],
                                trashB[:, j : j + 512],
                                start=False,
                                stop=True,
                            )
                        nc.vector.tensor_copy(cs_sb[:, :], cs_ps[:, :])
                    else:
                        nc.vector.tensor_copy(cs_sb[:, :], E_r[0:1, :])
                    # masked sims sums (independent of the collectives) are
                    # computed here, off the post-RS critical path
                    nc.vector.tensor_mul(trashB[:, :], sims_r[:, :], mask[:, :])
                    nc.vector.reduce_sum(ssum_v[:, :], trashB[:, :], axis=AX.X)
                    nc.vector.tensor_scalar(
                        ssum_v[:, :], ssum_v[:, :], scale_b, None, ALU.mult
                    )
                    nc.vector.tensor_mul(trashB[:, :], simsT_c[:, :], mask[:, :])
                    nc.vector.reduce_sum(ssum_t[:, :], trashB[:, :], axis=AX.X)
                    nc.vector.tensor_scalar(
                        ssum_t[:, :], ssum_t[:, :], scale_b, None, ALU.mult
                    )

            # ---------- queue grind ----------
            # Per row tile: 4 chunks of 2048 matmul columns land in PSUM
            # (double buffered).  3 chunks are copied by DVE into an SBUF
            # staging tile and exp'd in ONE wide ACT instruction (amortizes
            # the per-instruction ACT overhead); the 4th chunk is exp'd
            # directly from PSUM (in place) so ACT and DVE loads balance
            # (ACT ~0.88ns/elem staged + 1 chunk direct vs DVE 1.13ns/elem
            # on the staged 3/4 of the data).

            A32_trick = (8388608.0 / LN2) * scale_q

            def grind_direct(queue_sb, lhsT, qsum, pg, est_pool, cc_ap=None,
                             accAg=None, accDg=None):
                # v2: dual-engine grind.  Per row tile, 8 sub-chunks of 1024
                # matmul cols land in their own [128,1024] PSUM tiles
                # (bufs=4); ~56% drain on ACT (exact exp, accum in accA),
                # ~44% on DVE (trick-convert into an f32 stage, then one
                # fused pair-sum with accum into accD).
                for r in range(RT):
                    lhs = _f32r(lhsT[:, r * 128 : (r + 1) * 128])
                    act_scs = ACT_SC_EVEN if r % 2 == 0 else ACT_SC_ODD
                    ndve = NSC - len(act_scs)
                    stage = None
                    if ndve:
                        stage = est_pool.tile([128, 4096], F32, tag="stage")
                    accA = accAg[:, r * 8 : r * 8 + 8]
                    accD = accDg[:, r : r + 1]
                    di = 0
                    for sc in range(NSC):
                        ps = pg.tile([128, SC], F32, tag="gps")
                        for j in range(2):
                            col = sc * SC + j * 512
                            nc.tensor.matmul(
                                ps[:, j * 512 : (j + 1) * 512],
                                lhs,
                                queue_sb[:, col : col + 512],
                                start=True,
                                stop=True,
                            )
                        if sc in act_scs:
                            k = act_scs.index(sc)
                            nc.scalar.activation(
                                ps[:, :],
                                ps[:, :],
                                AF.Exp,
                                scale=scale_q,
                                accum_out=accA[:, k : k + 1],
                            )
                        else:
                            nc.vector.tensor_scalar(
                                stage[:, di * SC : (di + 1) * SC].bitcast(I32),
                                ps[:, :],
                                A32_trick,
                                B32_TRICK,
                                ALU.mult,
                                ALU.add,
                            )
                            di += 1
                    if ndve:
                        used = di * SC
                        h = used // 2
                        nc.vector.scalar_tensor_tensor(
                            p2trash[:, 0:h],
                            stage[:, 0:h],
                            1.0,
                            stage[:, h:used],
                            ALU.mult,
                            ALU.add,
                            accum_out=accD,
                        )
                # deferred combine: keeps the DVE stream free of waits on
                # ACT's accumulators during the grind
                for r in range(RT):
                    nc.vector.reduce_sum(
                        qsum[:, r : r + 1], accAg[:, r * 8 : r * 8 + 8],
                        axis=AX.X,
                    )
                nc.vector.tensor_add(qsum[:, :], qsum[:, :], accDg[:, :])
                if cc_ap is not None:
                    for r in range(RT):
                        nc.sync.dma_start(out=cc_ap[r], in_=qsum[:, r : r + 1])

            grind = grind_direct

            if bench_loops > 0:
                # benchmark mode: repeat both grinds inside a HW loop; the
                # grinds are idempotent so results stay correct.
                assert stage >= 8
                with (
                    tc.tile_pool(name="pgb", bufs=4, space="PSUM") as pg,
                    tc.tile_pool(name="estb", bufs=2) as estp,
                ):
                    with tc.For_i(0, bench_loops, 1):
                        grind(tq_sb, vfT, qsum_v, pg, estp,
                              accAg=accAg_v, accDg=accDg_v)
                        grind(vq_sb, tfT, qsum_t, pg, estp,
                              accAg=accAg_t, accDg=accDg_t)
            elif stage >= 5:
                # text queue -> qsum_v (feeds RS2)
                with (
                    tc.tile_pool(name="pgv", bufs=4, space="PSUM") as pg,
                    tc.tile_pool(name="estv", bufs=2) as estp,
                ):
                    cc2aps = (
                        [cc2_in.ap()[r, :] for r in range(RT)]
                        if stage >= 6
                        else None
                    )
                    grind(tq_sb, vfT, qsum_v, pg, estp, cc2aps,
                          accAg=accAg_v, accDg=accDg_v)

            if stage >= 6:
                nc.gpsimd.collective_compute(
                    "ReduceScatter",
                    ALU.add,
                    replica_groups=rg,
                    ins=[cc2_in.ap().opt()],
                    outs=[cc2_out.ap().opt()],
                )

            if stage >= 7:
                # vision queue -> qsum_t (feeds RS1)
                if bench_loops == 0:
                    with (
                        tc.tile_pool(name="pgt", bufs=4, space="PSUM") as pg,
                        tc.tile_pool(name="estt", bufs=2) as estp,
                    ):
                        cc1aps = [cc1_in.ap()[r, 0, :] for r in range(RT)]
                        grind(vq_sb, tfT, qsum_t, pg, estp, cc1aps,
                              accAg=accAg_t, accDg=accDg_t)
                for r in range(RT):
                    if bench_loops != 0:
                        nc.sync.dma_start(
                            out=cc1_in.ap()[r, 0, :], in_=qsum_t[:, r : r + 1]
                        )
                    nc.sync.dma_start(
                        out=cc1_in.ap()[r, 1, :],
                        in_=cs_sb[0:1, r * 128 : (r + 1) * 128],
                    )
                nc.gpsimd.collective_compute(
                    "ReduceScatter",
                    ALU.add,
                    replica_groups=rg,
                    ins=[cc1_in.ap().opt()],
                    outs=[cc1_out.ap().opt()],
                )

            if stage >= 8:
                # ---------- phase D: loss terms for own shard ----------
                with tc.tile_pool(name="psD", bufs=1, space="PSUM") as psD:
                    # v2t: rows shard.  neg_v = batch-nonmatch rowsum + queue
                    nc.sync.dma_start(out=qvt[:, :], in_=cc2_out.ap()[0, :])
                    nc.vector.tensor_add(negv[:, :], rnm[:, :], qvt[:, :])
                    nc.scalar.activation(
                        _f32r(sqbuf[:, :]), E_r[:, :], AF.Ln, bias=negv[:, 0:1]
                    )
                    nc.vector.tensor_mul(trashB[:, :], sqbuf[:, :], mask[:, :])
                    nc.vector.reduce_sum(lsum_v[:, :], trashB[:, :], axis=AX.X)
                    nc.vector.tensor_sub(lv[:, :], lsum_v[:, :], ssum_v[:, :])

                    # t2v: cols shard.  neg_t = batch colsum + queue sum
                    nc.sync.dma_start(out=cst[:, :], in_=cc1_out.ap()[1, :])
                    nc.sync.dma_start(out=qtt[:, :], in_=cc1_out.ap()[0, :])
                    nc.vector.tensor_add(negt[:, :], cst[:, :], qtt[:, :])
                    nc.scalar.activation(
                        _f32r(sqbuf[:, :]), ET_c[:, :], AF.Ln, bias=negt[:, 0:1]
                    )
                    nc.vector.tensor_mul(trashB[:, :], sqbuf[:, :], mask[:, :])
                    nc.vector.reduce_sum(lsum_t[:, :], trashB[:, :], axis=AX.X)
                    nc.vector.tensor_sub(lt[:, :], lsum_t[:, :], ssum_t[:, :])

                # ---------- outputs ----------
                nc.sync.dma_start(out=out_d.ap()[:, 0:1], in_=lv[:, :])
                nc.sync.dma_start(out=out_d.ap()[:, 1:2], in_=lt[:, :])
                nc.sync.dma_start(out=out_d.ap()[:, 2:3], in_=np_rows[:, :])
            else:
                # debug stages: emit whatever is defined
                nc.sync.dma_start(out=out_d.ap()[:, 0:1], in_=np_rows[:, :])
                src1 = E_r if stage >= 2 else np_rows
                nc.sync.dma_start(out=out_d.ap()[:, 1:2], in_=src1[:, 0:1])
                src2 = qsum_v if stage >= 5 else np_rows
                nc.sync.dma_start(out=out_d.ap()[:, 2:3], in_=src2[:, 0:1])

    nc.compile()
    return nc


def schedule_scalars(fill_level: int):
    fill_ratio = min(int(fill_level), Q) / Q
    eff_temp = MAX_TEMP - (MAX_TEMP - INIT_TEMP) * fill_ratio
    if fill_ratio >= 0.95:
        eff_temp = INIT_TEMP
    queue_weight = min(1.0, fill_ratio * 1.5)
    if fill_ratio < 0.2:
        queue_weight = fill_ratio * 0.5
    return eff_temp, queue_weight


def make_in_maps(
    vision_features, text_features, match_ids, vision_queue, text_queue
):
    vf = np.asarray(vision_features, dtype=np.float32)
    tf_ = np.asarray(text_features, dtype=np.float32)
    vq = np.asarray(vision_queue, dtype=np.float32)
    tq = np.asarray(text_queue, dtype=np.float32)
    mid = np.asarray(match_ids).astype(np.float32)

    vfT = np.ascontiguousarray(vf.T)
    tfT = np.ascontiguousarray(tf_.T)
    mid_bcast = np.ascontiguousarray(np.broadcast_to(mid.reshape(1, B), (128, B)))

    in_maps = []
    for k in range(NCORES):
        rk = slice(k * 128, (k + 1) * 128)
        qs = slice(k * QS, (k + 1) * QS)
        in_maps.append(
            {
                "vfT": vfT,
                "tfT": tfT,
                "vf_rkT": np.ascontiguousarray(vf[rk].T),
                "tf_rkT": np.ascontiguousarray(tf_[rk].T),
                "mid": mid_bcast,
                "mid_rk": np.ascontiguousarray(mid[rk].reshape(128, 1)),
                "tq": np.ascontiguousarray(tq[:, qs]),
                "vq": np.ascontiguousarray(vq[:, qs]),
            }
        )
    return in_maps


def combine_partials(partials_list):
    """partials_list: NCORES arrays of [128, 3] -> scalar loss (fp32)."""
    P = np.stack([np.asarray(p, dtype=np.float64) for p in partials_list])
    s = P.sum(axis=(0, 1))  # [3] = (v2t, t2v, num_pos)
    loss = (s[0] / s[2] + s[1] / s[2]) / 2.0
    return np.float32(loss)


_NC_CACHE: dict = {}


def _get_compiled(eff_temp: float, queue_weight: float, stage: int = 8):
    key = (round(eff_temp, 9), round(queue_weight, 9), stage)
    if key not in _NC_CACHE:
        _NC_CACHE[key] = build(eff_temp, queue_weight, stage=stage)
    return _NC_CACHE[key]


def kernel(
    vision_features,
    text_features,
    match_ids,
    vision_queue,
    text_queue,
    fill_level,
    **_ignored,
):
    eff_temp, queue_weight = schedule_scalars(fill_level)
    nc = _get_compiled(eff_temp, queue_weight)
    in_maps = make_in_maps(
        vision_features, text_features, match_ids, vision_queue, text_queue
    )
    res = bass_utils.run_bass_kernel_spmd(
        nc, in_maps, core_ids=list(range(NCORES))
    )
    return combine_partials([r["partials"] for r in res.results])



# revision 9
# speedup vs baseline: 1.3371x; 1.0477x over previous
"""Trainium2 Bass kernel for nn_MemoryQueueContrastiveLoss.

Strategy (8 NeuronCores):
  - Shard the QUEUE dimension (65536 -> 8 x 8192) across cores; replicate the
    batch features.  Each core computes partial queue negative sums
    (sum_q exp(s/t)) for ALL 1024 batch rows over its queue shard, plus the
    batch-vs-batch part for its own 128-row / 128-col shard.
  - Two ReduceScatter collectives combine the per-core partial sums so core k
    receives exactly its row-shard slice of the global negative sums.
  - Each core then computes its shard of the final loss terms
      log1p(neg * exp(-s)) = ln(exp(s) + neg) - s
    and returns per-partition partial sums; the host adds 8x[128] partials.

All transcendentals (exp/ln) run on the ACT engine, which is the bottleneck
(~2*B*Q/8 = 16.8M exps/core).  Matmuls run as float32r (full PE rate).
"""

import sys

for _p in ("/opt/trn_rl_repo",):
    if _p not in sys.path:
        sys.path.insert(0, _p)

import numpy as np

import concourse.bass as bass  # noqa: F401  (registers types)
import concourse.bacc as bacc
import concourse.mybir as mybir
from concourse import tile
from concourse import bass_utils

B = 1024          # batch
D = 128           # feature dim
Q = 65536         # queue size
NCORES = 8
QS = Q // NCORES  # 8192 queue columns per core
RT = B // 128     # 8 row tiles
INIT_TEMP = 0.07
MAX_TEMP = 0.07 * 1.3

F32 = mybir.dt.float32
F32R = mybir.dt.float32r
I32 = mybir.dt.int32
AF = mybir.ActivationFunctionType
ALU = mybir.AluOpType
AX = mybir.AxisListType

# ACT tile width for the queue exp grind: 2048 fp32 = 4 PSUM banks.
GW = 2048
NG = QS // GW     # 4 grind chunks per row tile
NMM = GW // 512   # 4 matmuls per grind chunk

# v2 dual-engine grind: 8 sub-chunks of 1024 queue cols per row tile.
# ACT consumes 5 (even row tiles) / 4 (odd) sub-chunks with exact
# exp+accumulate; DVE consumes the rest via a Schraudolph bit-trick
# (affine + f32->i32 convert writes the bit pattern of ~exp(z), then one
# fused pair-sum tensor_scalar with accum).  Engines get DISJOINT PSUM
# tiles and accumulator tiles so their streams never serialize.
SC = 1024                  # sub-chunk width
NSC = QS // SC             # 8 sub-chunks per row tile
import os as _os_cfg
_ACT_MODE = _os_cfg.environ.get("KSPLIT", "mix")
if _ACT_MODE == "act":
    ACT_SC_EVEN = tuple(range(8))
    ACT_SC_ODD = tuple(range(8))
elif _ACT_MODE == "dve":
    ACT_SC_EVEN = ()
    ACT_SC_ODD = ()
else:
    ACT_SC_EVEN = (0, 2, 4, 6, 7)
    ACT_SC_ODD = (0, 2, 4, 6)
LN2 = 0.6931471805599453
# f32 Schraudolph bias with mean-error correction (-482870 ~= -0.0576 oct)
B32_TRICK = 1064870346.0


def _f32r(ap):
    return ap.bitcast(F32R)


def build(
    eff_temp: float,
    queue_weight: float,
    n_cores: int = NCORES,
    stage: int = 8,
    bench_loops: int = 0,
):
    """Emit + compile the SPMD program (same program on all cores).

    stage (debug bisect): 1=DMA+norms, 2=+sims matmul/exp, 3=+exp accum,
    4=+full phase B, 5=+text grind, 6=+RS2, 7=+vision grind+RS1, 8=full.
    """
    scale_b = 1.0 / eff_temp            # batch sims logits scale
    scale_q = queue_weight / eff_temp   # queue logits scale

    nc = bacc.Bacc(
        "TRN2", target_bir_lowering=False, debug=False, num_devices=n_cores
    )

    # ---- kernel I/O (per core) ----
    vfT_d = nc.dram_tensor("vfT", [D, B], F32R, kind="ExternalInput")
    tfT_d = nc.dram_tensor("tfT", [D, B], F32R, kind="ExternalInput")
    vfrkT_d = nc.dram_tensor("vf_rkT", [D, 128], F32R, kind="ExternalInput")
    tfrkT_d = nc.dram_tensor("tf_rkT", [D, 128], F32R, kind="ExternalInput")
    mid_d = nc.dram_tensor("mid", [128, B], F32, kind="ExternalInput")
    midrk_d = nc.dram_tensor("mid_rk", [128, 1], F32, kind="ExternalInput")
    tq_d = nc.dram_tensor("tq", [D, QS], F32R, kind="ExternalInput")
    vq_d = nc.dram_tensor("vq", [D, QS], F32R, kind="ExternalInput")
    out_d = nc.dram_tensor("partials", [128, 3], F32, kind="ExternalOutput")

    # ---- collective buffers (internal DRAM) ----
    # cc2: qsum_v partials, laid out [row_tile, lane] so ReduceScatter hands
    # core k the summed block for its own row shard.
    cc2_in = nc.dram_tensor("cc2_in", [RT, 128], F32)
    cc2_out = nc.dram_tensor("cc2_out", [1, 128], F32)
    # cc1: [row_tile, 2, lane] = (qsum_t, batch colsum) partials.
    cc1_in = nc.dram_tensor("cc1_in", [RT, 2, 128], F32)
    cc1_out = nc.dram_tensor("cc1_out", [2, 128], F32)

    rg = [list(range(n_cores))]

    with tile.TileContext(nc) as tc:
        with tc.tile_pool(name="sb", bufs=1) as sb:
            # persistent SBUF tiles
            vfT = sb.tile([D, B], F32R, tag="vfT")
            tfT = sb.tile([D, B], F32R, tag="tfT")
            vfrkT = sb.tile([D, 128], F32R, tag="vfrkT")
            tfrkT = sb.tile([D, 128], F32R, tag="tfrkT")
            midb = sb.tile([128, B], F32, tag="midb")
            midrk = sb.tile([128, 1], F32, tag="midrk")
            tq_sb = sb.tile([D, QS], F32R, tag="tq")
            vq_sb = sb.tile([D, QS], F32R, tag="vq")
            mask = sb.tile([128, B], F32, tag="mask")
            sqbuf = sb.tile([128, B], F32, tag="sqbuf")
            lnbuf = sb.tile([1, B], F32, tag="lnbuf")
            rnbuf = sb.tile([1, B], F32, tag="rnbuf")
            ones = sb.tile([128, 1], F32, tag="ones")
            nones = sb.tile([128, 1], F32, tag="nones")
            ones1 = sb.tile([1, 128], F32R, tag="ones1")
            ones1f = sb.tile([1, 128], F32, tag="ones1f")
            ones_r = sb.tile([128, 1], F32R, tag="ones_r")
            E_r = sb.tile([128, B], F32, tag="E_r")
            ET_c = sb.tile([128, B], F32, tag="ET_c")
            rsumE = sb.tile([128, 1], F32, tag="rsumE")
            possum = sb.tile([128, 1], F32, tag="possum")
            rnm = sb.tile([128, 1], F32, tag="rnm")
            cs_sb = sb.tile([1, B], F32, tag="cs_sb")
            np_rows = sb.tile([128, 1], F32, tag="np_rows")
            qsum_v = sb.tile([128, RT], F32, tag="qsum_v")
            qsum_t = sb.tile([128, RT], F32, tag="qsum_t")
            p2trash = sb.tile([128, 2048], F32, tag="p2trash")
            acttrash = sb.tile([128, SC], F32, tag="acttrash")
            accAg_v = sb.tile([128, RT * 8], F32, tag="accAg_v")
            accDg_v = sb.tile([128, RT], F32, tag="accDg_v")
            accAg_t = sb.tile([128, RT * 8], F32, tag="accAg_t")
            accDg_t = sb.tile([128, RT], F32, tag="accDg_t")
            trashB = sb.tile([128, B], F32, tag="trashB")
            qvt = sb.tile([128, 1], F32, tag="qvt")
            qtt = sb.tile([128, 1], F32, tag="qtt")
            cst = sb.tile([128, 1], F32, tag="cst")
            negv = sb.tile([128, 1], F32, tag="negv")
            negt = sb.tile([128, 1], F32, tag="negt")
            lsum_v = sb.tile([128, 1], F32, tag="lsum_v")
            lsum_t = sb.tile([128, 1], F32, tag="lsum_t")
            ssum_v = sb.tile([128, 1], F32, tag="ssum_v")
            ssum_t = sb.tile([128, 1], F32, tag="ssum_t")
            lv = sb.tile([128, 1], F32, tag="lv")
            lt = sb.tile([128, 1], F32, tag="lt")

            # ---------- input DMAs ----------
            nc.sync.dma_start(out=vfT[:, :], in_=vfT_d.ap()[:, :])
            nc.sync.dma_start(out=tfT[:, :], in_=tfT_d.ap()[:, :])
            nc.sync.dma_start(out=vfrkT[:, :], in_=vfrkT_d.ap()[:, :])
            nc.sync.dma_start(out=tfrkT[:, :], in_=tfrkT_d.ap()[:, :])
            nc.sync.dma_start(out=midb[:, :], in_=mid_d.ap()[:, :])
            nc.sync.dma_start(out=midrk[:, :], in_=midrk_d.ap()[:, :])
            # queue shards, chunked so compute can start early
            for c in range(NG):
                cs_ = slice(c * GW, (c + 1) * GW)
                nc.sync.dma_start(out=tq_sb[:, cs_], in_=tq_d.ap()[:, cs_])
            for c in range(NG):
                cs_ = slice(c * GW, (c + 1) * GW)
                nc.sync.dma_start(out=vq_sb[:, cs_], in_=vq_d.ap()[:, cs_])

            nc.vector.memset(accAg_v[:, :], 0.0)
            nc.vector.memset(accDg_v[:, :], 0.0)
            nc.vector.memset(accAg_t[:, :], 0.0)
            nc.vector.memset(accDg_t[:, :], 0.0)
            nc.vector.memset(ones[:, :], 1.0)
            nc.vector.memset(nones[:, :], -1.0)
            nc.vector.memset(ones1f[:, :], 1.0)
            nc.vector.tensor_copy(ones1[:, :], ones1f[:, :])
            nc.vector.tensor_copy(ones_r[:, :], ones[:, :])

            # ---------- phase A: l2-normalize features (in place) ----------
            def norm_chain(xT, n, psA):
                nc.vector.tensor_mul(_f32r(sqbuf[:, :n]), xT[:, :], xT[:, :])
                n2 = psA.tile([1, B], F32, tag="n2")
                for j in range(0, n, 512):
                    nc.tensor.matmul(
                        n2[:, j : j + 512],
                        ones_r[:, :],
                        _f32r(sqbuf[:, j : j + 512]),
                        start=True,
                        stop=True,
                    )
                # rnorm = exp(-0.5 * ln(norm2))  (avoids sqrt table load)
                nc.scalar.activation(lnbuf[:, :n], n2[:, :n], AF.Ln)
                nc.scalar.activation(
                    _f32r(rnbuf[:, :n]), lnbuf[:, :n], AF.Exp, scale=-0.5
                )
                # broadcast rnorm across partitions via PE: ones1^T @ rnorm_row
                rb = psA.tile([128, B], F32, tag="rb")
                for j in range(0, n, 512):
                    nc.tensor.matmul(
                        rb[:, j : j + 512],
                        ones1[0:1, :],
                        _f32r(rnbuf[0:1, j : j + 512]),
                        start=True,
                        stop=True,
                    )
                # write the normalized features as float32r so the verifier
                # accepts them as fp32r-matmul inputs
                nc.vector.tensor_mul(_f32r(xT[:, :]), xT[:, :], rb[:, :n])

            with tc.tile_pool(name="psA", bufs=2, space="PSUM") as psA:
                norm_chain(vfT, B, psA)   # vision first: text-queue grind needs it
                norm_chain(tfT, B, psA)
                norm_chain(vfrkT, 128, psA)
                norm_chain(tfrkT, 128, psA)

            # match mask for this core's row/col shard: mask[p, j] =
            # (mid[rk_p] == mid[j])
            nc.vector.tensor_scalar(
                mask[:, :], midb[:, :], midrk[:, 0:1], None, ALU.is_equal
            )
            nc.vector.reduce_sum(np_rows[:, :], mask[:, :], axis=AX.X)

            # ---------- phase B: batch sims for own shard ----------
            if stage >= 2:
                with tc.tile_pool(name="psB", bufs=1, space="PSUM") as psB:
                    sims_r = psB.tile([128, B], F32, tag="sims_r")
                    simsT_c = psB.tile([128, B], F32, tag="simsT_c")
                    cs_ps = psB.tile([1, B], F32, tag="cs_ps")
                    for j in range(0, B, 512):
                        nc.tensor.matmul(
                            sims_r[:, j : j + 512],
                            _f32r(vfrkT[:, :]),
                            _f32r(tfT[:, j : j + 512]),
                            start=True,
                            stop=True,
                        )
                    nc.scalar.activation(
                        E_r[:, :],
                        sims_r[:, :],
                        AF.Exp,
                        scale=scale_b,
                        accum_out=rsumE[:, :] if stage >= 3 else None,
                    )
                    for j in range(0, B, 512):
                        nc.tensor.matmul(
                            simsT_c[:, j : j + 512],
                            _f32r(tfrkT[:, :]),
                            _f32r(vfT[:, j : j + 512]),
                            start=True,
                            stop=True,
                        )
                    nc.scalar.activation(
                        ET_c[:, :], simsT_c[:, :], AF.Exp, scale=scale_b
                    )

                    import os as _os

                    _sub = int(_os.environ.get("KSUB", "9"))
                    if stage >= 4 and _sub >= 1:
                        # Em = E_r * mask ; possum = rowsum(Em)
                        nc.vector.tensor_mul(trashB[:, :], E_r[:, :], mask[:, :])
                        nc.vector.reduce_sum(possum[:, :], trashB[:, :], axis=AX.X)
                        nc.vector.tensor_sub(rnm[:, :], rsumE[:, :], possum[:, :])
                    if stage >= 4 and _sub >= 2:
                        # batch colsums of non-matching exp(sims)
                        for j in range(0, B, 512):
                            nc.tensor.matmul(
                                cs_ps[:, j : j + 512],
                                ones[:, :],
                                E_r[:, j : j + 512],
                                start=True,
                                stop=False,
                            )
                            nc.tensor.matmul(
                                cs_ps[:, j : j + 512],
                                nones[:, :],
                                trashB[:, j : j + 512],
                                start=False,
                                stop=True,
                            )
                        nc.vector.tensor_copy(cs_sb[:, :], cs_ps[:, :])
                    else:
                        nc.vector.tensor_copy(cs_sb[:, :], E_r[0:1, :])
                    # masked sims sums (independent of the collectives) are
                    # computed here, off the post-RS critical path
                    nc.vector.tensor_mul(trashB[:, :], sims_r[:, :], mask[:, :])
                    nc.vector.reduce_sum(ssum_v[:, :], trashB[:, :], axis=AX.X)
                    nc.vector.tensor_scalar(
                        ssum_v[:, :], ssum_v[:, :], scale_b, None, ALU.mult
                    )
                    nc.vector.tensor_mul(trashB[:, :], simsT_c[:, :], mask[:, :])
                    nc.vector.reduce_sum(ssum_t[:, :], trashB[:, :], axis=AX.X)
                    nc.vector.tensor_scalar(
                        ssum_t[:, :], ssum_t[:, :], scale_b, None, ALU.mult
                    )

            # ---------- queue grind ----------
            # Per row tile: 4 chunks of 2048 matmul columns land in PSUM
            # (double buffered).  3 chunks are copied by DVE into an SBUF
            # staging tile and exp'd in ONE wide ACT instruction (amortizes
            # the per-instruction ACT overhead); the 4th chunk is exp'd
            # directly from PSUM (in place) so ACT and DVE loads balance
            # (ACT ~0.88ns/elem staged + 1 chunk direct vs DVE 1.13ns/elem
            # on the staged 3/4 of the data).

            A32_trick = (8388608.0 / LN2) * scale_q

            def grind_direct(queue_sb, lhsT, qsum, pg, est_pool, cc_ap=None,
                             accAg=None, accDg=None):
                # v2: dual-engine grind.  Per row tile, 8 sub-chunks of 1024
                # matmul cols land in their own [128,1024] PSUM tiles
                # (bufs=4); ~56% drain on ACT (exact exp, accum in accA),
                # ~44% on DVE (trick-convert into an f32 stage, then one
                # fused pair-sum with accum into accD).
                for r in range(RT):
                    lhs = _f32r(lhsT[:, r * 128 : (r + 1) * 128])
                    act_scs = ACT_SC_EVEN if r % 2 == 0 else ACT_SC_ODD
                    ndve = NSC - len(act_scs)
                    stage = None
                    if ndve:
                        stage = est_pool.tile([128, 4096], F32, tag="stage")
                    accA = accAg[:, r * 8 : r * 8 + 8]
                    accD = accDg[:, r : r + 1]
                    di = 0
                    for sc in range(NSC):
                        ps = pg.tile([128, SC], F32, tag="gps")
                        for j in range(2):
                            col = sc * SC + j * 512
                            nc.tensor.matmul(
                                ps[:, j * 512 : (j + 1) * 512],
                                lhs,
                                queue_sb[:, col : col + 512],
                                start=True,
                                stop=True,
                            )
                        if sc in act_scs:
                            k = act_scs.index(sc)
                            # main output goes to an SBUF scratch tile: the
                            # exp values are dead, only the accum is used;
                            # writing them back to PSUM would burn PSUM
                            # access bandwidth shared with PE and DVE
                            nc.scalar.activation(
                                acttrash[:, :],
                                ps[:, :],
                                AF.Exp,
                                scale=scale_q,
                                accum_out=accA[:, k : k + 1],
                            )
                        else:
                            nc.vector.tensor_scalar(
                                stage[:, di * SC : (di + 1) * SC].bitcast(I32),
                                ps[:, :],
                                A32_trick,
                                B32_TRICK,
                                ALU.mult,
                                ALU.add,
                            )
                            di += 1
                    if ndve:
                        used = di * SC
                        h = used // 2
                        nc.vector.scalar_tensor_tensor(
                            p2trash[:, 0:h],
                            stage[:, 0:h],
                            1.0,
                            stage[:, h:used],
                            ALU.mult,
                            ALU.add,
                            accum_out=accD,
                        )
                # deferred combine: keeps the DVE stream free of waits on
                # ACT's accumulators during the grind
                for r in range(RT):
                    nc.vector.reduce_sum(
                        qsum[:, r : r + 1], accAg[:, r * 8 : r * 8 + 8],
                        axis=AX.X,
                    )
                nc.vector.tensor_add(qsum[:, :], qsum[:, :], accDg[:, :])
                if cc_ap is not None:
                    for r in range(RT):
                        nc.sync.dma_start(out=cc_ap[r], in_=qsum[:, r : r + 1])

            grind = grind_direct

            if bench_loops > 0:
                # benchmark mode: repeat both grinds inside a HW loop; the
                # grinds are idempotent so results stay correct.
                assert stage >= 8
                with (
                    tc.tile_pool(name="pgb", bufs=4, space="PSUM") as pg,
                    tc.tile_pool(name="estb", bufs=2) as estp,
                ):
                    with tc.For_i(0, bench_loops, 1):
                        grind(tq_sb, vfT, qsum_v, pg, estp,
                              accAg=accAg_v, accDg=accDg_v)
                        grind(vq_sb, tfT, qsum_t, pg, estp,
                              accAg=accAg_t, accDg=accDg_t)
            elif stage >= 5:
                # text queue -> qsum_v (feeds RS2)
                with (
                    tc.tile_pool(name="pgv", bufs=4, space="PSUM") as pg,
                    tc.tile_pool(name="estv", bufs=2) as estp,
                ):
                    cc2aps = (
                        [cc2_in.ap()[r, :] for r in range(RT)]
                        if stage >= 6
                        else None
                    )
                    grind(tq_sb, vfT, qsum_v, pg, estp, cc2aps,
                          accAg=accAg_v, accDg=accDg_v)

            if stage >= 6:
                nc.gpsimd.collective_compute(
                    "ReduceScatter",
                    ALU.add,
                    replica_groups=rg,
                    ins=[cc2_in.ap().opt()],
                    outs=[cc2_out.ap().opt()],
                )

            if stage >= 7:
                # vision queue -> qsum_t (feeds RS1)
                if bench_loops == 0:
                    with (
                        tc.tile_pool(name="pgt", bufs=4, space="PSUM") as pg,
                        tc.tile_pool(name="estt", bufs=2) as estp,
                    ):
                        cc1aps = [cc1_in.ap()[r, 0, :] for r in range(RT)]
                        grind(vq_sb, tfT, qsum_t, pg, estp, cc1aps,
                              accAg=accAg_t, accDg=accDg_t)
                for r in range(RT):
                    if bench_loops != 0:
                        nc.sync.dma_start(
                            out=cc1_in.ap()[r, 0, :], in_=qsum_t[:, r : r + 1]
                        )
                    nc.sync.dma_start(
                        out=cc1_in.ap()[r, 1, :],
                        in_=cs_sb[0:1, r * 128 : (r + 1) * 128],
                    )
                nc.gpsimd.collective_compute(
                    "ReduceScatter",
                    ALU.add,
                    replica_groups=rg,
                    ins=[cc1_in.ap().opt()],
                    outs=[cc1_out.ap().opt()],
                )

            if stage >= 8:
                # ---------- phase D: loss terms for own shard ----------
                with tc.tile_pool(name="psD", bufs=1, space="PSUM") as psD:
                    # v2t: rows shard.  neg_v = batch-nonmatch rowsum + queue
                    nc.sync.dma_start(out=qvt[:, :], in_=cc2_out.ap()[0, :])
                    nc.vector.tensor_add(negv[:, :], rnm[:, :], qvt[:, :])
                    nc.scalar.activation(
                        _f32r(sqbuf[:, :]), E_r[:, :], AF.Ln, bias=negv[:, 0:1]
                    )
                    nc.vector.tensor_mul(trashB[:, :], sqbuf[:, :], mask[:, :])
                    nc.vector.reduce_sum(lsum_v[:, :], trashB[:, :], axis=AX.X)
                    nc.vector.tensor_sub(lv[:, :], lsum_v[:, :], ssum_v[:, :])

                    # t2v: cols shard.  neg_t = batch colsum + queue sum
                    nc.sync.dma_start(out=cst[:, :], in_=cc1_out.ap()[1, :])
                    nc.sync.dma_start(out=qtt[:, :], in_=cc1_out.ap()[0, :])
                    nc.vector.tensor_add(negt[:, :], cst[:, :], qtt[:, :])
                    nc.scalar.activation(
                        _f32r(sqbuf[:, :]), ET_c[:, :], AF.Ln, bias=negt[:, 0:1]
                    )
                    nc.vector.tensor_mul(trashB[:, :], sqbuf[:, :], mask[:, :])
                    nc.vector.reduce_sum(lsum_t[:, :], trashB[:, :], axis=AX.X)
                    nc.vector.tensor_sub(lt[:, :], lsum_t[:, :], ssum_t[:, :])

                # ---------- outputs ----------
                nc.sync.dma_start(out=out_d.ap()[:, 0:1], in_=lv[:, :])
                nc.sync.dma_start(out=out_d.ap()[:, 1:2], in_=lt[:, :])
                nc.sync.dma_start(out=out_d.ap()[:, 2:3], in_=np_rows[:, :])
            else:
                # debug stages: emit whatever is defined
                nc.sync.dma_start(out=out_d.ap()[:, 0:1], in_=np_rows[:, :])
                src1 = E_r if stage >= 2 else np_rows
                nc.sync.dma_start(out=out_d.ap()[:, 1:2], in_=src1[:, 0:1])
                src2 = qsum_v if stage >= 5 else np_rows
                nc.sync.dma_start(out=out_d.ap()[:, 2:3], in_=src2[:, 0:1])

    nc.compile()
    return nc


def schedule_scalars(fill_level: int):
    fill_ratio = min(int(fill_level), Q) / Q
    eff_temp = MAX_TEMP - (MAX_TEMP - INIT_TEMP) * fill_ratio
    if fill_ratio >= 0.95:
        eff_temp = INIT_TEMP
    queue_weight = min(1.0, fill_ratio * 1.5)
    if fill_ratio < 0.2:
        queue_weight = fill_ratio * 0.5
    return eff_temp, queue_weight


def make_in_maps(
    vision_features, text_features, match_ids, vision_queue, text_queue
):
    vf = np.asarray(vision_features, dtype=np.float32)
    tf_ = np.asarray(text_features, dtype=np.float32)
    vq = np.asarray(vision_queue, dtype=np.float32)
    tq = np.asarray(text_queue, dtype=np.float32)
    mid = np.asarray(match_ids).astype(np.float32)

    vfT = np.ascontiguousarray(vf.T)
    tfT = np.ascontiguousarray(tf_.T)
    mid_bcast = np.ascontiguousarray(np.broadcast_to(mid.reshape(1, B), (128, B)))

    in_maps = []
    for k in range(NCORES):
        rk = slice(k * 128, (k + 1) * 128)
        qs = slice(k * QS, (k + 1) * QS)
        in_maps.append(
            {
                "vfT": vfT,
                "tfT": tfT,
                "vf_rkT": np.ascontiguousarray(vf[rk].T),
                "tf_rkT": np.ascontiguousarray(tf_[rk].T),
                "mid": mid_bcast,
                "mid_rk": np.ascontiguousarray(mid[rk].reshape(128, 1)),
                "tq": np.ascontiguousarray(tq[:, qs]),
                "vq": np.ascontiguousarray(vq[:, qs]),
            }
        )
    return in_maps


def combine_partials(partials_list):
    """partials_list: NCORES arrays of [128, 3] -> scalar loss (fp32)."""
    P = np.stack([np.asarray(p, dtype=np.float64) for p in partials_list])
    s = P.sum(axis=(0, 1))  # [3] = (v2t, t2v, num_pos)
    loss = (s[0] / s[2] + s[1] / s[2]) / 2.0
    return np.float32(loss)


_NC_CACHE: dict = {}


def _get_compiled(eff_temp: float, queue_weight: float, stage: int = 8):
    key = (round(eff_temp, 9), round(queue_weight, 9), stage)
    if key not in _NC_CACHE:
        _NC_CACHE[key] = build(eff_temp, queue_weight, stage=stage)
    return _NC_CACHE[key]


def kernel(
    vision_features,
    text_features,
    match_ids,
    vision_queue,
    text_queue,
    fill_level,
    **_ignored,
):
    eff_temp, queue_weight = schedule_scalars(fill_level)
    nc = _get_compiled(eff_temp, queue_weight)
    in_maps = make_in_maps(
        vision_features, text_features, match_ids, vision_queue, text_queue
    )
    res = bass_utils.run_bass_kernel_spmd(
        nc, in_maps, core_ids=list(range(NCORES))
    )
    return combine_partials([r["partials"] for r in res.results])

